# revision 21
# baseline (speedup 1.0000x reference)
"""GCN autoencoder (2-layer GCN + inner-product decoder) on 8 Trainium2
NeuronCores.

recon = A@(relu(A@(X W1)) W2) ; out = Z Z^T   with A[dst,src] += edge_w.

Sharding: nodes are split 1024-per-core. Each core holds TWO dense bf16
slices of A^T: the column slice A^T[:, dst_i] (streamed, used to form
H1 for its nodes) and the row slice A^T[src_i, :] (resident, used to
form its partial contribution to the full Z). Layer 1+2 are fused and
streamed: XW1 is computed per 128-node tile and immediately consumed by
the H1 accumulation, so the 16 MB A^T column slice never stays in SBUF.
Each core then computes partial_Z^T = Y_i^T @ A^T[src_i, :] over ALL
destination nodes; one ReduceScatter sums the partials on the wire and
hands each core its own Z^T columns, and one AllGather (fed directly
from the ReduceScatter output buffer) shares Z^T with everyone.
Finally each core emits its (1024, 8192) block of Z Z^T through
full-row 4 MB output DMAs.

All matmuls run in bf16 with fp32 PSUM accumulation; the collectives
carry bf16 (a tiny warmup AllGather absorbs the CC-stream wakeup cost).
"""

import os

import ml_dtypes
import numpy as np

N_NODES = 8192
N_CORES = 8
ROWS = N_NODES // N_CORES          # 1024 output rows per core
KT = N_NODES // 128                # 64 node k-tiles globally
KTM = ROWS // 128                  # 8 node k-tiles per core
D_IN, D_H, D_Z = 512, 256, 64

BF16 = ml_dtypes.bfloat16

_COMPILED = None        # cached (nc, meta) across kernel() calls
LAST_EXEC_TIME_NS = None
LAST_RESULTS = None


def _patch_tile_drain():
    """This container's walrus build rejects instructions carrying more
    than one sync-wait command (it lacks the multi-wait lowering).  Two
    fixes, both inside Tile's teardown:

    1. A legalization pass over every traced instruction: extra waits
       are hoisted onto fresh single-wait nops inserted just before the
       instruction on the same engine (same-engine sequencing preserves
       semantics).
    2. The kernel-tail drain (one wait per live semaphore) is split the
       same way.
    """
    import concourse.mybir as mybir
    import concourse.tile as tile
    from bass_rust import ScopedClock

    def _split_multi_waits(nc):
        f = nc.m.functions[0]
        for blk in f.blocks:
            insts = list(blk.instructions)
            if not any(
                i.sync_info is not None and len(i.sync_info.on_wait) > 1
                for i in insts
            ):
                continue
            new_list = []
            for inst in insts:
                si = inst.sync_info
                if si is not None and len(si.on_wait) > 1 and inst.engine in nc.engines:
                    waits = list(si.on_wait)
                    eng = nc.engines[inst.engine]
                    for w in waits[:-1]:
                        n = eng.nop(nofuse=True, hint="wsplit")
                        # the builder appended it to cur_bb; relocate
                        cb = nc.cur_bb.bb
                        cur = cb.instructions
                        assert cur and cur[-1].name == n.ins.name
                        cur.pop()
                        cb.instructions = cur
                        n.ins.sync_info = mybir.SyncInfo(
                            on_wait=[w], on_update=[]
                        )
                        new_list.append(n.ins)
                    inst.sync_info = mybir.SyncInfo(
                        on_wait=[waits[-1]], on_update=list(si.on_update)
                    )
                new_list.append(inst)
            blk.instructions = new_list

    def _drain_and_barrier(self, tick_clock, wait_clock):
        nc = self.nc
        _split_multi_waits(nc)
        probe = nc.sync.nop(nofuse=True, hint="drain_waits")
        wait_clock.add_sem_waits(
            probe.ins, ScopedClock({None: tick_clock.global_clock})
        )
        waits = list(probe.ins.sync_info.on_wait) if probe.ins.sync_info else []
        if len(waits) > 1:
            probe.ins.sync_info = mybir.SyncInfo(on_wait=[waits[0]], on_update=[])
            for w in waits[1:]:
                n = nc.sync.nop(nofuse=True, hint="drain_waits")
                n.ins.sync_info = mybir.SyncInfo(on_wait=[w], on_update=[])
        nc.sync.drain()
        nc.all_engine_barrier()
        assert self.sems is not None
        popped = nc._tile_sem_poison_stack.pop()
        assert popped is self._sem_poison
        nc.clear_and_free_semaphores(list(self.sems.allocated().values()))
        nc.all_engine_barrier()

    tile.TileContext._drain_and_barrier = _drain_and_barrier


def _build_program():
    import concourse.bass as bass
    import concourse.mybir as mybir
    import concourse.tile as tile

    _patch_tile_drain()

    nc = bass.Bass(num_devices=N_CORES)
    bf = mybir.dt.bfloat16
    f32 = mybir.dt.float32

    # A^T column slice [src, my dst], streamed:  [128, KT, ROWS]
    atc_in = nc.dram_tensor("atc_in", [128, KT, ROWS], bf, kind="ExternalInput")
    # A^T row slice [my src, all dst], resident: [128, KTM, N]
    atr_in = nc.dram_tensor("atr_in", [128, KTM, N_NODES], bf,
                            kind="ExternalInput")
    xt_in = nc.dram_tensor("xt_in", [128, D_IN // 128, N_NODES], bf,
                           kind="ExternalInput")
    w1_in = nc.dram_tensor("w1_in", [128, D_IN // 128, D_H], bf,
                           kind="ExternalInput")
    w2_in = nc.dram_tensor("w2_in", [128, D_H // 128, D_Z], bf,
                           kind="ExternalInput")
    recon_out = nc.dram_tensor("recon", [ROWS, N_NODES], f32,
                               kind="ExternalOutput")

    groups = [list(range(N_CORES))]
    MB = ROWS // 128   # 8 row blocks per core

    with tile.TileContext(nc) as tc:
        with (
            tc.tile_pool(name="dram", bufs=1, space="DRAM") as dram,
            tc.tile_pool(name="res", bufs=1) as res,
        ):
            # ---- collective bounce buffers (DRAM), split in column
            # halves so the A2A/AG pipeline overlaps compute ----
            HC = ROWS // 2   # 512 columns per stage-4 pass
            a2a_in_d = dram.tile([N_CORES, D_Z, ROWS], bf, name="a2a_in_d")
            rs_out_d = dram.tile([D_Z, ROWS], bf, name="rs_out_d")
            ztf_d = dram.tile([N_CORES, D_Z, ROWS], bf, addr_space="Shared",
                              name="ztf_d")
            warm_in_d = dram.tile([N_CORES, 16], bf, name="warm_in_d")
            warm_out_d = dram.tile([N_CORES * N_CORES, 16], bf,
                                   addr_space="Shared", name="warm_out_d")

            # ---- resident SBUF ----
            h1t_sb = res.tile([128, D_H // 128, ROWS], bf, name="h1t_sb")
            y_sb = res.tile([128, KTM, D_Z], bf, name="y_sb")
            w1_sb = res.tile([128, D_IN // 128, D_H], bf, name="w1_sb")
            w2_sb = res.tile([128, D_H // 128, D_Z], bf, name="w2_sb")
            ztm2_sb = res.tile([D_Z, ROWS], bf, name="ztm2_sb")
            ztf2_sb = res.tile([D_Z, N_CORES, ROWS], bf, name="ztf2_sb")
            bias_sb = res.tile([128, 1], f32, name="bias_sb")
            warm_sb = res.tile([N_CORES, 16], bf, name="warm_sb")
            nc.vector.memset(bias_sb[:], 0.0)
            nc.vector.memset(warm_sb[:], 0.0)
            nc.sync.dma_start(out=w1_sb[:], in_=w1_in[:])
            nc.sync.dma_start(out=w2_sb[:], in_=w2_in[:])
            nc.sync.dma_start(out=warm_in_d[:], in_=warm_sb[:])
            # tiny dummy collective: pays the first-trigger wakeup cost
            # of the CC stream while stage 1 computes
            nc.gpsimd.collective_compute(
                "AllGather", mybir.AluOpType.bypass, replica_groups=groups,
                ins=[warm_in_d[:]], outs=[warm_out_d[:]],
            )

            atrp = ctx_atr = tc.tile_pool(name="atrp", bufs=1)
            atrp = ctx_atr.__enter__()
            atr_sb = atrp.tile([128, KTM, N_NODES], bf, name="atr_sb")  # 128KB/p
            # row-slice load: overlaps the whole fused stage 1+2
            for r in range(KTM):
                nc.sync.dma_start(out=atr_sb[:, r, :], in_=atr_in[:, r, :])

            # ---- fused stage 1+2: stream xt and A^T cols per k-tile ----
            CH = 4          # k-tiles per streamed A^T chunk
            with (
                tc.tile_pool(name="xts", bufs=2) as xts,
                tc.tile_pool(name="ats", bufs=3) as ats,
                tc.tile_pool(name="xw1r", bufs=8) as xw1r,
                tc.tile_pool(name="ps1", bufs=2, space="PSUM") as ps1,
                tc.tile_pool(name="ps2", bufs=1, space="PSUM") as ps2,
            ):
                # S2 accumulators [128, 512] f32 live across all 64 k-tiles.
                s2acc = [
                    ps2.tile([128, 512], f32, name=f"s2acc{fb}{nb}")
                    for fb in range(D_H // 128) for nb in range(ROWS // 512)
                ]
                xt_t = None
                for c in range(KT // CH):
                    at_t = ats.tile([128, CH, ROWS], bf, tag="at", name="at_t")
                    nc.sync.dma_start(
                        out=at_t[:], in_=atc_in[:, c * CH:(c + 1) * CH, :])
                    if c % 2 == 0:
                        xt_t = xts.tile([128, D_IN // 128, 1024], bf,
                                        tag="xt", name="xt_t")
                        nc.sync.dma_start(
                            out=xt_t[:],
                            in_=xt_in[:, :, (c // 2) * 1024:(c // 2 + 1) * 1024])
                    for kk in range(CH):
                        k = c * CH + kk
                        noff = (k % (1024 // 128)) * 128
                        acc1 = ps1.tile([128, D_H], f32, tag="s1", name="acc1")
                        for f in range(D_IN // 128):
                            nc.tensor.matmul(
                                acc1[:],
                                xt_t[:, f, noff:noff + 128],
                                w1_sb[:, f, :],
                                start=(f == 0),
                                stop=(f == D_IN // 128 - 1),
                            )
                        xw1_t = xw1r.tile([128, D_H], bf, tag="xw1",
                                          name="xw1_t")
                        if k % 2 == 0:
                            nc.vector.tensor_copy(xw1_t[:], acc1[:])
                        else:
                            nc.scalar.activation(
                                xw1_t[:], acc1[:],
                                mybir.ActivationFunctionType.Copy)
                        for fb in range(D_H // 128):
                            for nb in range(ROWS // 512):
                                nc.tensor.matmul(
                                    s2acc[fb * 2 + nb][:],
                                    xw1_t[:, fb * 128:(fb + 1) * 128],
                                    at_t[:, kk, nb * 512:(nb + 1) * 512],
                                    start=(k == 0),
                                    stop=(k == KT - 1),
                                )

                for fb in range(D_H // 128):
                    for nb in range(ROWS // 512):
                        nc.scalar.activation(
                            h1t_sb[:, fb, nb * 512:(nb + 1) * 512],
                            s2acc[fb * 2 + nb][:],
                            mybir.ActivationFunctionType.Relu, bias=bias_sb[:],
                        )

                # ---- stage 3: Y_i = H1_i @ W2  (node-major) ----
                for b in range(MB):
                    accy = ps1.tile([128, D_Z], f32, tag="s1", name="accy")
                    for fk in range(D_H // 128):
                        nc.tensor.matmul(
                            accy[:],
                            h1t_sb[:, fk, b * 128:(b + 1) * 128],
                            w2_sb[:, fk, :],
                            start=(fk == 0),
                            stop=(fk == D_H // 128 - 1),
                        )
                    nc.vector.tensor_copy(y_sb[:, b, :], accy[:])

            # ---- stage 4: partial Z^T over ALL dst, in two column-half
            # passes; each pass feeds its own AllToAll so collective #0
            # overlaps the pass-1 matmuls ----
            with (
                tc.tile_pool(name="zs", bufs=1) as zs,
                tc.tile_pool(name="zst", bufs=1) as zst,
                tc.tile_pool(name="ps4", bufs=1, space="PSUM") as ps4,
            ):
                for p in range(2):
                    # two [64, 2048] accs cover ranks 0-3 / 4-7's half-cols
                    acca = ps4.tile([D_Z, 4 * HC], f32, tag="s4a",
                                    name="acca")
                    accb = ps4.tile([D_Z, 4 * HC], f32, tag="s4b",
                                    name="accb")
                    for j in range(N_CORES):
                        acc = acca if j < 4 else accb
                        s = j % 4
                        col0 = j * ROWS + p * HC
                        for k in range(KTM):
                            nc.tensor.matmul(
                                acc[:, s * HC:(s + 1) * HC],
                                y_sb[:, k, :],
                                atr_sb[:, k, col0:col0 + HC],
                                start=(k == 0),
                                stop=(k == KTM - 1),
                            )
                    za = zst.tile([D_Z, 4 * HC], bf, tag="za", name="za")
                    zb = zst.tile([D_Z, 4 * HC], bf, tag="zb", name="zb")
                    nc.vector.tensor_copy(za[:], acca[:])
                    nc.scalar.activation(zb[:], accb[:],
                                         mybir.ActivationFunctionType.Copy)
                    for j in range(N_CORES):
                        src = za if j < 4 else zb
                        nc.sync.dma_start(
                            out=a2a_in_d[j][:, p * HC:(p + 1) * HC],
                            in_=src[:, (j % 4) * HC:(j % 4 + 1) * HC])
                # ReduceScatter sums the 8 partial slabs on the wire and
                # hands each core exactly its own columns of Z^T
                nc.gpsimd.collective_compute(
                    "ReduceScatter", mybir.AluOpType.add, replica_groups=groups,
                    ins=[a2a_in_d[:]], outs=[rs_out_d[:]],
                )
                nc.gpsimd.collective_compute(
                    "AllGather", mybir.AluOpType.bypass, replica_groups=groups,
                    ins=[rs_out_d[:]], outs=[ztf_d[:]],
                )
                nc.sync.dma_start(out=ztm2_sb[:], in_=rs_out_d[:])
                for j in range(N_CORES):
                    nc.sync.dma_start(out=ztf2_sb[:, j, :], in_=ztf_d[j])

            ctx_atr.__exit__(None, None, None)

            # ---- stage 5: recon_i = Z_i @ Z^T, f32 out, full-row DMAs ----
            with (
                tc.tile_pool(name="outp", bufs=3) as outp,
                tc.tile_pool(name="ps5", bufs=2, space="PSUM") as ps5,
            ):
                for b in range(MB):
                    ot = outp.tile([128, N_NODES], f32, tag="out", name="ot")
                    n0 = b * 128
                    for g in range(4):
                        acc = ps5.tile([128, 2048], f32, tag="s5", name="acc5")
                        for s in range(4):
                            col = g * 2048 + s * 512
                            j, off = divmod(col, ROWS)
                            nc.tensor.matmul(
                                acc[:, s * 512:(s + 1) * 512],
                                ztm2_sb[:, n0:n0 + 128],
                                ztf2_sb[:, j, off:off + 512],
                                start=True, stop=True,
                            )
                            if s % 2 == 0:
                                nc.vector.tensor_copy(
                                    ot[:, col:col + 512],
                                    acc[:, s * 512:(s + 1) * 512])
                            else:
                                nc.scalar.activation(
                                    ot[:, col:col + 512],
                                    acc[:, s * 512:(s + 1) * 512],
                                    mybir.ActivationFunctionType.Copy)
                    nc.sync.dma_start(
                        out=recon_out[b * 128:(b + 1) * 128, :],
                        in_=ot[:],
                    )

    nc.finalize()
    return nc


def _prep_inputs(x, edge_src, edge_dst, edge_w, W1, W2):
    """Host-side: dense A^T, per-core slices, PE-friendly bf16 layouts."""
    at = np.zeros((N_NODES, N_NODES), dtype=np.float32)     # [src, dst]
    np.add.at(at, (edge_src, edge_dst), edge_w)

    xt = np.ascontiguousarray(x.T)                          # [feat, node]
    xt_dev = np.ascontiguousarray(
        xt.reshape(D_IN // 128, 128, N_NODES).transpose(1, 0, 2)).astype(BF16)
    w1_dev = np.ascontiguousarray(
        W1.reshape(D_IN // 128, 128, D_H).transpose(1, 0, 2)).astype(BF16)
    w2_dev = np.ascontiguousarray(
        W2.reshape(D_H // 128, 128, D_Z).transpose(1, 0, 2)).astype(BF16)

    in_maps = []
    for i in range(N_CORES):
        rows = slice(i * ROWS, (i + 1) * ROWS)
        atc_i = np.ascontiguousarray(
            at[:, rows].reshape(KT, 128, ROWS).transpose(1, 0, 2)).astype(BF16)
        atr_i = np.ascontiguousarray(
            at[rows, :].reshape(KTM, 128, N_NODES).transpose(1, 0, 2)
        ).astype(BF16)
        in_maps.append({
            "atc_in": atc_i, "atr_in": atr_i, "xt_in": xt_dev,
            "w1_in": w1_dev, "w2_in": w2_dev,
        })
    return in_maps


def kernel(x, edge_src, edge_dst, edge_w, W1, W2):
    global _COMPILED, LAST_EXEC_TIME_NS, LAST_RESULTS
    from concourse.bass_utils import run_bass_kernel_spmd

    if _COMPILED is None:
        _COMPILED = _build_program()
    nc = _COMPILED

    in_maps = _prep_inputs(
        np.asarray(x, dtype=np.float32),
        np.asarray(edge_src), np.asarray(edge_dst),
        np.asarray(edge_w, dtype=np.float32),
        np.asarray(W1, dtype=np.float32), np.asarray(W2, dtype=np.float32),
    )

    trace = bool(int(os.environ.get("KERNEL_TRACE", "0")))
    res = run_bass_kernel_spmd(
        nc, in_maps, list(range(N_CORES)), trace=trace,
    )
    LAST_RESULTS = res
    LAST_EXEC_TIME_NS = res.exec_time_ns
    return np.concatenate([res.results[i]["recon"] for i in range(N_CORES)], axis=0)
